# revision 30
# baseline (speedup 1.0000x reference)
"""Fused linear + cross-entropy loss (BaseChunkLoss) on 8 trn2 NeuronCores.

Strategy (vocab/tensor parallel, per the sharding hint's second option):
  - head_weight is sharded over vocab: each core owns a 4000-column slice
    of W (and bias) and computes, for ALL 8192 tokens, the partial
    sum-of-exp over its vocab slice.  The cross-device "logsumexp" is the
    host-side combine: s[tok] = sum_c s_c[tok], lse = log(s).
  - Per-core HBM traffic is ~3x lower than token-sharding (hidden 67MB +
    W slice 33MB vs. full W 262MB), which moves the kernel from DMA-bound
    to PE-bound at the fp8 DoubleRow matmul floor (~426us/core).
  - f32 -> fp8 conversion of both hidden and W happens inside the casting
    software-DGE DMA (gpsimd queue), so no staging buffers or on-chip
    conversion passes are needed.  W is cast unscaled; e4m3 subnormal
    rounding gives per-element absolute error comparable to the pre-scaled
    variant (measured end-to-end loss rel-err ~3e-5).
  - The target logit is computed exactly in f32 as a row-dot of the
    host-gathered rows W[labels] with hidden (token-sharded: each core
    does its own 1024 tokens), as mul+reduce chunks slotted into the
    DVE drain slack.  Host adds bias[labels] during the combine.

Device layout: tokens on PSUM partitions, vocab on the free dim.
  lhsT = hidden^T fp8 tile [128 d-pairs x 2 x 128 tok]  (stationary)
  rhs  = W^T      fp8 tile [128 d-pairs x 2 x 500 vocab] (moving)
  psum [128 tok x 4 banks x 500 vocab] f32, accumulated over D=2048 in 8
  DoubleRow steps (K=256 per matmul).
Drain per psum tile: DVE writes junk_bf16 = psum + bias (this frees the
psum bank pair in ~2.5us, under the 3.3us PE fill time, so PE never
stalls), then ACT computes exp(junk) with a fused row-sum accumulator
into s_cols, off the critical path.

Host-side prep is layout-only (transpose/slice/gather); all FLOPs over
hidden/weights happen on device inside the measured kernel.  The final
scalar combine (log, weighted mean, 8-way add) is the wrapper's
all_reduce stand-in.
"""
import numpy as np
from contextlib import ExitStack

from concourse import bacc, mybir, tile
from concourse.bass_utils import run_bass_kernel_spmd

F32 = mybir.dt.float32
BF16 = mybir.dt.bfloat16
FP8 = mybir.dt.float8e4
Alu = mybir.AluOpType
Act = mybir.ActivationFunctionType

N_CORES = 8
N_TOK = 8192
D = 2048
V = 32000
P = 128

VC = V // N_CORES       # 4000 vocab columns per core
GV = 2000               # vocab columns per drain group
NG = VC // GV           # 2 groups
BANKS = 4               # psum banks per group
BANK = GV // BANKS      # 500 vocab columns per bank
KP = D // 256           # 8 DoubleRow contraction steps (K=256 each)
TB = 1024               # tokens per streamed hidden block
NB = N_TOK // TB        # 8 blocks
MBB = TB // P           # 8 psum m-blocks per hidden block
MG = N_TOK // P         # 64 global m-blocks (s output columns)
TC = N_TOK // N_CORES   # 1024 tokens per core for the exact row-dot
MR = TC // P            # 8 row-dot m-blocks

# startup gpsimd DMA order: ("w", (lo, hi)) | ("h", (lo, hi)) | ("b", group)
STARTUP_ORDER = (
    ("w", (0, GV // 2)), ("h", (0, TB)), ("b", 0), ("w", (GV // 2, GV)),
    ("b", 1), ("w", (GV, VC)),
)


def _build():
    nc = bacc.Bacc("TRN2", target_bir_lowering=False, debug=False)
    # h and W arrive pre-transposed from host: h [D, N], W [D, VC]
    h_d = nc.declare_dram_parameter("h", [D, N_TOK], F32, isOutput=False)
    W_d = nc.declare_dram_parameter("W", [D, VC], F32, isOutput=False)
    bias_d = nc.declare_dram_parameter("bias", [VC], F32, isOutput=False)
    hn_d = nc.declare_dram_parameter("hn", [TC, D], F32, isOutput=False)
    wg_d = nc.declare_dram_parameter("wg", [TC, D], F32, isOutput=False)
    s_out = nc.declare_dram_parameter("s_out", [P, MG], F32, isOutput=True)
    t_out = nc.declare_dram_parameter("t_out", [P, MR], F32, isOutput=True)

    W_r = W_d[:].rearrange("(kp j ki) v -> kp ki j v", ki=P, j=2)  # [8,128,2,VC]
    h_r = h_d[:].rearrange("(kp j ki) t -> kp ki j t", ki=P, j=2)  # [8,128,2,N]

    with tile.TileContext(nc) as tc, ExitStack() as ctx:
        wpool = ctx.enter_context(tc.tile_pool(name="w", bufs=1))
        hpool = ctx.enter_context(tc.tile_pool(name="hT", bufs=2))
        bpool = ctx.enter_context(tc.tile_pool(name="bias", bufs=1))
        gpool = ctx.enter_context(tc.tile_pool(name="gath", bufs=2))
        dpool = ctx.enter_context(tc.tile_pool(name="dj", bufs=1))
        jpool = ctx.enter_context(tc.tile_pool(name="jt", bufs=3))
        epool = ctx.enter_context(tc.tile_pool(name="et", bufs=2))
        pspool = ctx.enter_context(tc.tile_pool(name="ps", bufs=2, space="PSUM"))
        acc = ctx.enter_context(tc.tile_pool(name="acc", bufs=1))

        # cols 0:128 = (m, g) accumulators; 128:136 = block-0 first-group
        # second-half partials; 136 = last-tile second-half partial
        s_cols = acc.tile([P, MG * NG + MBB + 1], F32, tag="scols")
        s_fin = acc.tile([P, MG], F32, tag="sfin")
        t_fin = acc.tile([P, MR], F32, tag="tfin")
        tpart = acc.tile([P, MR * 4], F32, tag="tpart")
        DC = D // 4             # row-dot chunk width

        wv = wpool.tile([P, KP, 2, VC], FP8, tag="w")
        bb = bpool.tile([P, VC], F32, tag="bias")

        def load_h(b):
            ht = hpool.tile([P, KP, 2, TB], FP8, tag="hT")
            for kp in range(KP):
                nc.gpsimd.dma_start(
                    ht[:, kp, :, :], h_r[kp][:, :, b * TB:(b + 1) * TB])
            return ht

        # Issue order on the gpsimd queue == DMA order.  The first psum
        # tile needs W group 0 + the first 128 tokens of hidden + bias
        # group 0; everything else overlaps with compute.
        ht_cur = hpool.tile([P, KP, 2, TB], FP8, tag="hT")

        def _dma_w(lo, hi):
            for kp in range(KP):
                nc.gpsimd.dma_start(
                    wv[:, kp, :, lo:hi], W_r[kp][:, :, lo:hi])

        def _dma_h0(lo, hi):
            for kp in range(KP):
                nc.gpsimd.dma_start(
                    ht_cur[:, kp, :, lo:hi], h_r[kp][:, :, lo:hi])

        def _dma_bias(g):
            nc.gpsimd.dma_start(
                bb[:, g * GV:(g + 1) * GV],
                bias_d[g * GV:(g + 1) * GV].partition_broadcast(P))

        # Startup DMA order (gpsimd queue == transfer order).  Chosen by
        # simulator sweep; the first psum tile needs W group 0 + the first
        # 128 tokens of hidden + bias group 0.
        for step in STARTUP_ORDER:
            kind, a = step
            if kind == "w":
                _dma_w(*a)
            elif kind == "h":
                _dma_h0(*a)
            else:
                _dma_bias(a)

        def half_tile_pass(ht, pt, voff, msubs, cols, fuse=False):
            """Two 2-bank sub-tiles (one per (m, vocab-half)) sharing one
            4-bank psum tile: matmul sweeps first, then the drains.  With
            fuse=True (both subs over the SAME vocab half for different
            m-blocks) the bias-add runs as one 4-bank DVE op against a
            stride-0-repeated bias view, keeping the DVE drain rate equal
            to the main loop's."""
            def _mm_sub(i, m, bk0):
                for kp in range(KP):
                    lhsT = ht[:, kp, :, m * P:(m + 1) * P]
                    for bk in range(2):
                        nc.tensor.matmul(
                            pt[:, 2 * i + bk, 0:BANK], lhsT,
                            wv[:, kp, :,
                               voff + (bk0 + bk) * BANK:
                               voff + (bk0 + bk + 1) * BANK],
                            start=(kp == 0), stop=(kp == KP - 1),
                            perf_mode=mybir.MatmulPerfMode.DoubleRow)

            def _drain_sub(i, cols):
                bk0 = msubs[i][1]
                bb2 = bb[:, voff + bk0 * BANK:voff + (bk0 + 2) * BANK]
                jt = jpool.tile([P, 2, BANK], BF16, tag="jt2")
                nc.vector.tensor_tensor(
                    jt[:], pt[:, 2 * i:2 * i + 2, 0:BANK],
                    bb2.rearrange("p (k c) -> p k c", c=BANK), op=Alu.add)
                et = epool.tile([P, 2, BANK], BF16, tag="et2")
                nc.scalar.activation(
                    et[:], jt[:], Act.Exp,
                    accum_out=s_cols[:, cols[i]:cols[i] + 1])

            for i, (m, bk0) in enumerate(msubs):
                _mm_sub(i, m, bk0)
            if not fuse:
                _drain_sub(0, cols)
                _drain_sub(1, cols)
                return
            if fuse:
                bk0 = msubs[0][1]
                bbv4 = bb[:, voff + bk0 * BANK:voff + (bk0 + 2) * BANK] \
                    .rearrange("p (o k c) -> p o k c", o=1, c=BANK) \
                    .broadcast_to([P, 2, 2, BANK])
                jt = jpool.tile([P, BANKS, BANK], BF16, tag="jt")
                nc.vector.tensor_tensor(
                    jt[:].rearrange("p (a k) c -> p a k c", a=2),
                    pt[:, 0:BANKS, 0:BANK].rearrange(
                        "p (a k) c -> p a k c", a=2),
                    bbv4, op=Alu.add)
                et = epool.tile([P, BANKS, BANK], BF16, tag="et")
                for i in range(2):
                    nc.scalar.activation(
                        et[:, 2 * i:2 * i + 2, :], jt[:, 2 * i:2 * i + 2, :],
                        Act.Exp, accum_out=s_cols[:, cols[i]:cols[i] + 1])

        # Exact-f32 target-logit row-dot, chopped into D/4-wide chunks that
        # slot into the per-drain DVE slack (PE fills a psum tile in 3.33us,
        # the drain takes 2.2us; each 0.7us chunk fits the gap).  The
        # multiply chunks for block b run during b's g1 drains; the reduce
        # chunks run during block b+1's g0 drains (half-block lag so the
        # hg/wg DMAs always arrive in time).
        dj_prev = None
        for b in range(NB):
            ht = ht_cur
            hg = gpool.tile([P, D], F32, tag="hg")
            nc.gpsimd.dma_start(hg[:], hn_d[b * P:(b + 1) * P, :])
            wgt = gpool.tile([P, D], F32, tag="wg")
            nc.gpsimd.dma_start(wgt[:], wg_d[b * P:(b + 1) * P, :])
            ht_next = load_h(b + 1) if b + 1 < NB else None
            dj = dpool.tile([P, D], F32, tag="dj")

            for g in range(NG):
                if b == 0 and g == 0:
                    # Block 0's first group runs as half-width (1000-col)
                    # sub-tiles so PE can start after only the first 1000
                    # W columns have arrived.  The second half accumulates
                    # into scratch cols 128+m, folded in during block 1.
                    for half in range(2):
                        for mp in range(MBB // 2):
                            pt = pspool.tile([P, BANKS, 512], F32, tag="ps")
                            ms = [(2 * mp, 2 * half), (2 * mp + 1, 2 * half)]
                            cols = [(2 * mp + s) * NG if half == 0
                                    else MG * NG + 2 * mp + s
                                    for s in range(2)]
                            half_tile_pass(ht, pt, 0, ms, cols, fuse=True)
                    continue
                bbv = bb[:, g * GV:(g + 1) * GV].rearrange(
                    "p (k c) -> p k c", c=BANK)
                for m in range(MBB):
                    last = b == NB - 1
                    if last and g == 1 and m == MBB - 1:
                        # last tile: bank-major halves so the closing drain
                        # is half-size (shorter tail chain); second half
                        # accumulates into scratch col 136.
                        pt = pspool.tile([P, BANKS, 512], F32, tag="ps")
                        col = (b * MBB + m) * NG + g
                        half_tile_pass(ht, pt, GV, [(m, 0), (m, 2)],
                                       [col, MG * NG + MBB])
                        continue
                    pt = pspool.tile([P, BANKS, 512], F32, tag="ps")
                    for kp in range(KP):
                        lhsT = ht[:, kp, :, m * P:(m + 1) * P]
                        for bk in range(BANKS):
                            nc.tensor.matmul(
                                pt[:, bk, 0:BANK], lhsT,
                                wv[:, kp, :,
                                   g * GV + bk * BANK:g * GV + (bk + 1) * BANK],
                                start=(kp == 0), stop=(kp == KP - 1),
                                perf_mode=mybir.MatmulPerfMode.DoubleRow)
                    jt = jpool.tile([P, BANKS, BANK], BF16, tag="jt")
                    nc.vector.tensor_tensor(
                        jt[:], pt[:, 0:BANKS, 0:BANK], bbv, op=Alu.add)
                    et = epool.tile([P, BANKS, BANK], BF16, tag="et")
                    col = (b * MBB + m) * NG + g
                    nc.scalar.activation(
                        et[:], jt[:], Act.Exp,
                        accum_out=s_cols[:, col:col + 1])
                    # row-dot chunks in the drain slack.  Normally: muls of
                    # block b in b's g1 slots, reduces in b+1's g0 slots.
                    # The last block pulls both into its own slots so the
                    # tail has no row-dot work left.
                    last = b == NB - 1
                    if g == 0 and m < 4 and dj_prev is not None:
                        c = slice(m * DC, (m + 1) * DC)
                        nc.vector.tensor_reduce(
                            tpart[:, (b - 1) * 4 + m:(b - 1) * 4 + m + 1],
                            dj_prev[:, c], axis=mybir.AxisListType.X,
                            op=Alu.add)
                    if (g == 0 and 4 <= m if last else g == 1 and m < 4):
                        mm = m - 4 if last else m
                        c = slice(mm * DC, (mm + 1) * DC)
                        nc.vector.tensor_mul(dj[:, c], hg[:, c], wgt[:, c])
                    if b == 1 and g == 1 and m == 4:
                        # fold block-0 first-group second-half partials
                        # (scratch cols) into their (m, g0) accumulators
                        sv0 = s_cols[:, 0:MG * NG].rearrange(
                            "p (m g) -> p m g", g=NG)
                        nc.vector.tensor_tensor(
                            sv0[:, 0:MBB, 0], sv0[:, 0:MBB, 0],
                            s_cols[:, MG * NG:MG * NG + MBB], op=Alu.add)
                    if last and g == 1 and m < 4:
                        c = slice(m * DC, (m + 1) * DC)
                        nc.vector.tensor_reduce(
                            tpart[:, b * 4 + m:b * 4 + m + 1],
                            dj[:, c], axis=mybir.AxisListType.X, op=Alu.add)
            dj_prev = dj
            ht_cur = ht_next

        tv = tpart[:].rearrange("p (m c) -> p m c", c=4)
        nc.vector.tensor_reduce(
            t_fin[:], tv, axis=mybir.AxisListType.X, op=Alu.add)
        nc.sync.dma_start(t_out[:], t_fin[:])
        sv = s_cols[:, 0:MG * NG].rearrange("p (m g) -> p m g", g=NG)
        nc.vector.tensor_tensor(s_fin[:], sv[:, :, 0], sv[:, :, 1], op=Alu.add)
        # last-tile second-half partial (scratch col 136)
        nc.vector.tensor_tensor(
            s_fin[:, MG - 1:MG], s_fin[:, MG - 1:MG],
            s_cols[:, MG * NG + MBB:MG * NG + MBB + 1], op=Alu.add)
        nc.sync.dma_start(s_out[:], s_fin[:])

    nc.compile()
    return nc


_NC_CACHE = {}


def _get_program():
    if "v2" not in _NC_CACHE:
        _NC_CACHE["v2"] = _build()
    return _NC_CACHE["v2"]


def kernel(hidden_states, head_weight, head_bias, loss_weight, labels,
           chunk_size=None, **_unused):
    hidden = np.asarray(hidden_states, dtype=np.float32)
    W = np.asarray(head_weight, dtype=np.float32)
    bias = np.asarray(head_bias, dtype=np.float32)
    lw = np.asarray(loss_weight, dtype=np.float32)
    labels = np.asarray(labels).astype(np.int64)

    assert hidden.shape == (N_TOK, D) and W.shape == (V, D)

    nc = _get_program()
    ht = np.ascontiguousarray(hidden.T)            # [D, N]
    Wt = np.ascontiguousarray(W.T)                 # [D, V]
    Wg = W[labels]                                 # gathered rows [N, D]
    in_maps = []
    for c in range(N_CORES):
        vsl = slice(c * VC, (c + 1) * VC)
        tsl = slice(c * TC, (c + 1) * TC)
        in_maps.append(dict(
            h=ht,
            W=np.ascontiguousarray(Wt[:, vsl]),
            bias=np.ascontiguousarray(bias[vsl]),
            hn=np.ascontiguousarray(hidden[tsl]),
            wg=np.ascontiguousarray(Wg[tsl])))
    res = run_bass_kernel_spmd(nc, in_maps, list(range(N_CORES)))

    # unshard + host-side scalar combine (the "all_reduce" of the hint):
    # sum the per-core partial exp-sums over vocab shards, then the
    # weighted-mean reduction over tokens.
    s = np.zeros((P, MG), dtype=np.float64)
    for r in res.results:
        s += r["s_out"].astype(np.float64)
    s = s.T.reshape(-1)                            # token-ordered [N]
    tgt = np.concatenate([r["t_out"].T.reshape(-1) for r in res.results])
    tgt = tgt.astype(np.float64) + bias[labels].astype(np.float64)
    lse = np.log(s)
    nll = lse - tgt
    w64 = lw.astype(np.float64)
    loss = (w64 * nll).sum() / max(w64.sum(), 1.0)
    return np.float32(loss)


# revision 57
# speedup vs baseline: 1.0205x; 1.0205x over previous
"""Fused linear + cross-entropy loss (BaseChunkLoss) on 8 trn2 NeuronCores.

Strategy (vocab/tensor parallel, per the sharding hint's second option):
  - head_weight is sharded over vocab: each core owns a 4000-column slice
    of W (and bias) and computes, for ALL 8192 tokens, the partial
    sum-of-exp over its vocab slice.  The cross-device "logsumexp" is the
    host-side combine: s[tok] = sum_c s_c[tok], lse = log(s).
  - Per-core HBM traffic is ~3x lower than token-sharding (hidden 67MB +
    W slice 33MB vs. full W 262MB), which moves the kernel from DMA-bound
    to PE-bound at the fp8 DoubleRow matmul floor (~426us/core).
  - f32 -> fp8 conversion of both hidden and W happens inside the casting
    software-DGE DMA (gpsimd queue), so no staging buffers or on-chip
    conversion passes are needed.  W is cast unscaled; e4m3 subnormal
    rounding gives per-element absolute error comparable to the pre-scaled
    variant (measured end-to-end loss rel-err ~3e-5).
  - The target logit is computed exactly in f32 as a row-dot of the
    host-gathered rows W[labels] with hidden (token-sharded: each core
    does its own 1024 tokens), as mul+reduce chunks slotted into the
    DVE drain slack.  Host adds bias[labels] during the combine.

Device layout: tokens on PSUM partitions, vocab on the free dim.
  lhsT = hidden^T fp8 tile [128 d-pairs x 2 x 128 tok]  (stationary)
  rhs  = W^T      fp8 tile [128 d-pairs x 2 x 500 vocab] (moving)
  psum [128 tok x 4 banks x 500 vocab] f32, accumulated over D=2048 in 8
  DoubleRow steps (K=256 per matmul).
Drain per psum tile: DVE writes junk_bf16 = psum + bias (this frees the
psum bank pair in ~2.5us, under the 3.3us PE fill time, so PE never
stalls), then ACT computes exp(junk) with a fused row-sum accumulator
into s_cols, off the critical path.

Host-side prep is layout-only (transpose/slice/gather); all FLOPs over
hidden/weights happen on device inside the measured kernel.  The final
scalar combine (log, weighted mean, 8-way add) is the wrapper's
all_reduce stand-in.
"""
import numpy as np
from contextlib import ExitStack

from concourse import bacc, mybir, tile
from concourse.bass_utils import run_bass_kernel_spmd

F32 = mybir.dt.float32
BF16 = mybir.dt.bfloat16
FP8 = mybir.dt.float8e4
Alu = mybir.AluOpType
Act = mybir.ActivationFunctionType

N_CORES = 8
N_TOK = 8192
D = 2048
V = 32000
P = 128

VC = V // N_CORES       # 4000 vocab columns per core
GV = 2000               # vocab columns per drain group
NG = VC // GV           # 2 groups
BANKS = 4               # psum banks per group
BANK = GV // BANKS      # 500 vocab columns per bank
KP = D // 256           # 8 DoubleRow contraction steps (K=256 each)
TB = 1024               # tokens per streamed hidden block
NB = N_TOK // TB        # 8 blocks
MBB = TB // P           # 8 psum m-blocks per hidden block
MG = N_TOK // P         # 64 global m-blocks (s output columns)
TC = N_TOK // N_CORES   # 1024 tokens per core for the exact row-dot
MR = TC // P            # 8 row-dot m-blocks

# startup gpsimd DMA order: ("w"|"h", (kp0, kp1, lo, hi)) | ("b", (lo, hi))
# | ("r", 0) for the block-0 row-dot inputs.  Fine-grained so PE's first
# 16-matmul sweep (512 tokens x 500 vocab x kp0-3) starts after ~4us of
# DMA; every chunk keeps the per-descriptor contiguous run >= 512B (below
# that DMA pays 2x latency).
STARTUP_ORDER = (
    ("w", (0, 4, 0, 512)), ("h", (0, 4, 0, 512)), ("b", (0, 500)),
    ("w", (4, 8, 0, 512)), ("h", (4, 8, 0, 512)),
    ("h", (0, 4, 512, TB)), ("h", (4, 8, 512, TB)),
    ("w", (0, 4, 512, 1024)), ("w", (4, 8, 512, 1024)), ("b", (500, 1000)),
    ("w", (0, 4, 1024, GV)), ("w", (4, 8, 1024, GV)), ("b", (1000, GV)),
    ("w", (0, 4, GV, VC)), ("w", (4, 8, GV, VC)), ("b", (GV, VC)),
    ("r", 0),
)

SC0 = MG * NG           # scratch accum cols: 32 for block-0 g0 quarters
SCL = SC0 + 32          # + 3 for the last m-block's banks 1-3
NSC = SCL + 3


def _build():
    nc = bacc.Bacc("TRN2", target_bir_lowering=False, debug=False)
    # h and W arrive pre-transposed from host: h [D, N], W [D, VC]
    h_d = nc.declare_dram_parameter("h", [D, N_TOK], F32, isOutput=False)
    W_d = nc.declare_dram_parameter("W", [D, VC], F32, isOutput=False)
    bias_d = nc.declare_dram_parameter("bias", [VC], F32, isOutput=False)
    hn_d = nc.declare_dram_parameter("hn", [TC, D], F32, isOutput=False)
    wg_d = nc.declare_dram_parameter("wg", [TC, D], F32, isOutput=False)
    # raw accumulator columns; the host combine does the final folds
    s_out = nc.declare_dram_parameter("s_out", [P, NSC], F32, isOutput=True)
    t_out = nc.declare_dram_parameter("t_out", [P, MR * 4], F32, isOutput=True)

    # kp-major 4D views: one DMA can cover several kp slices at once
    # (SWDGE desc-gen costs ~1us + 0.34ns/desc per DMA instruction, so
    # fewer/bigger DMAs keep the Pool desc-gen chain off the critical path)
    W_q = W_d[:].rearrange("(kp j ki) v -> ki kp j v", ki=P, j=2)  # [128,8,2,VC]
    h_q = h_d[:].rearrange("(kp j ki) t -> ki kp j t", ki=P, j=2)  # [128,8,2,N]

    with tile.TileContext(nc) as tc, ExitStack() as ctx:
        wpool = ctx.enter_context(tc.tile_pool(name="w", bufs=1))
        hpool = ctx.enter_context(tc.tile_pool(name="hT", bufs=2))
        bpool = ctx.enter_context(tc.tile_pool(name="bias", bufs=1))
        gpool = ctx.enter_context(tc.tile_pool(name="gath", bufs=2))
        dpool = ctx.enter_context(tc.tile_pool(name="dj", bufs=1))
        jpool = ctx.enter_context(tc.tile_pool(name="jt", bufs=3))
        epool = ctx.enter_context(tc.tile_pool(name="et", bufs=2))
        pspool = ctx.enter_context(tc.tile_pool(name="ps", bufs=2, space="PSUM"))
        acc = ctx.enter_context(tc.tile_pool(name="acc", bufs=1))

        # cols 0:128 = (m, g) accumulators; 128:160 = block-0 first-group
        # per-(m, bank) quarter partials (g0 slots m*2 stay unwritten for
        # m<8 -- host uses the quarters instead); 160 = last-tile second
        # half.  tpart holds the 4 row-dot chunk partials per m-block.
        s_cols = acc.tile([P, NSC], F32, tag="scols")
        tpart = acc.tile([P, MR * 4], F32, tag="tpart")
        DC = D // 4             # row-dot chunk width

        wv = wpool.tile([P, KP, 2, VC], FP8, tag="w")
        bb = bpool.tile([P, VC], F32, tag="bias")

        KC = 4                  # kp slices per DMA (1024 descriptors)

        def load_h(b):
            ht = hpool.tile([P, KP, 2, TB], FP8, tag="hT")
            for k0 in range(0, KP, KC):
                nc.gpsimd.dma_start(
                    ht[:, k0:k0 + KC, :, :],
                    h_q[:, k0:k0 + KC, :, b * TB:(b + 1) * TB])
            return ht

        # Issue order on the gpsimd queue == DMA order.  The first psum
        # tile needs W group 0 + the first 128 tokens of hidden + bias
        # group 0; everything else overlaps with compute.
        ht_cur = hpool.tile([P, KP, 2, TB], FP8, tag="hT")

        def _dma_w(k0, k1, lo, hi):
            nc.gpsimd.dma_start(
                wv[:, k0:k1, :, lo:hi], W_q[:, k0:k1, :, lo:hi])

        def _dma_h0(k0, k1, lo, hi):
            nc.gpsimd.dma_start(
                ht_cur[:, k0:k1, :, lo:hi], h_q[:, k0:k1, :, lo:hi])

        def _dma_bias(lo, hi):
            nc.gpsimd.dma_start(
                bb[:, lo:hi], bias_d[lo:hi].partition_broadcast(P))

        hg0 = wgt0 = None
        for step in STARTUP_ORDER:
            kind, a = step
            if kind == "w":
                _dma_w(*a)
            elif kind == "h":
                _dma_h0(*a)
            elif kind == "b":
                _dma_bias(*a)
            else:
                hg0 = gpool.tile([P, D], F32, tag="hg")
                nc.gpsimd.dma_start(hg0[:], hn_d[0:P, :])
                wgt0 = gpool.tile([P, D], F32, tag="wg")
                nc.gpsimd.dma_start(wgt0[:], wg_d[0:P, :])

        def half_tile_pass(ht, pt, voff, msubs, cols, fuse=False):
            """Two 2-bank sub-tiles (one per (m, vocab-half)) sharing one
            4-bank psum tile: matmul sweeps first, then the drains.  With
            fuse=True (both subs over the SAME vocab half for different
            m-blocks) the bias-add runs as one 4-bank DVE op against a
            stride-0-repeated bias view, keeping the DVE drain rate equal
            to the main loop's."""
            def _mm_sub(i, m, bk0):
                for kp in range(KP):
                    lhsT = ht[:, kp, :, m * P:(m + 1) * P]
                    for bk in range(2):
                        nc.tensor.matmul(
                            pt[:, 2 * i + bk, 0:BANK], lhsT,
                            wv[:, kp, :,
                               voff + (bk0 + bk) * BANK:
                               voff + (bk0 + bk + 1) * BANK],
                            start=(kp == 0), stop=(kp == KP - 1),
                            perf_mode=mybir.MatmulPerfMode.DoubleRow)

            def _drain_sub(i, cols):
                bk0 = msubs[i][1]
                bb2 = bb[:, voff + bk0 * BANK:voff + (bk0 + 2) * BANK]
                jt = jpool.tile([P, 2, BANK], BF16, tag="jt2")
                nc.vector.tensor_tensor(
                    jt[:], pt[:, 2 * i:2 * i + 2, 0:BANK],
                    bb2.rearrange("p (k c) -> p k c", c=BANK), op=Alu.add)
                et = epool.tile([P, 2, BANK], BF16, tag="et2")
                nc.scalar.activation(
                    et[:], jt[:], Act.Exp,
                    accum_out=s_cols[:, cols[i]:cols[i] + 1])

            for i, (m, bk0) in enumerate(msubs):
                _mm_sub(i, m, bk0)
            if not fuse:
                _drain_sub(0, cols)
                _drain_sub(1, cols)
                return
            if fuse:
                bk0 = msubs[0][1]
                bbv4 = bb[:, voff + bk0 * BANK:voff + (bk0 + 2) * BANK] \
                    .rearrange("p (o k c) -> p o k c", o=1, c=BANK) \
                    .broadcast_to([P, 2, 2, BANK])
                jt = jpool.tile([P, BANKS, BANK], BF16, tag="jt")
                nc.vector.tensor_tensor(
                    jt[:].rearrange("p (a k) c -> p a k c", a=2),
                    pt[:, 0:BANKS, 0:BANK].rearrange(
                        "p (a k) c -> p a k c", a=2),
                    bbv4, op=Alu.add)
                et = epool.tile([P, BANKS, BANK], BF16, tag="et")
                for i in range(2):
                    nc.scalar.activation(
                        et[:, 2 * i:2 * i + 2, :], jt[:, 2 * i:2 * i + 2, :],
                        Act.Exp, accum_out=s_cols[:, cols[i]:cols[i] + 1])

        # Exact-f32 target-logit row-dot, chopped into D/4-wide chunks that
        # slot into the per-drain DVE slack (PE fills a psum tile in 3.33us,
        # the drain takes 2.2us; each 0.7us chunk fits the gap).  The
        # multiply chunks for block b run during b's g1 drains; the reduce
        # chunks run during block b+1's g0 drains (half-block lag so the
        # hg/wg DMAs always arrive in time).
        dj_prev = None
        for b in range(NB):
            ht = ht_cur
            if b == 0:
                hg, wgt = hg0, wgt0
            else:
                hg = gpool.tile([P, D], F32, tag="hg")
                nc.gpsimd.dma_start(hg[:], hn_d[b * P:(b + 1) * P, :])
                wgt = gpool.tile([P, D], F32, tag="wg")
                nc.gpsimd.dma_start(wgt[:], wg_d[b * P:(b + 1) * P, :])
            ht_next = load_h(b + 1) if b + 1 < NB else None
            dj = dpool.tile([P, D], F32, tag="dj")

            for g in range(NG):
                if b == 0 and g == 0:
                    # Block 0's first group runs as quarter tiles: one psum
                    # tile hosts 4 m-blocks x 1 bank (500 vocab cols), so
                    # the first matmul sweep needs only the first 512 W
                    # columns + 512 tokens.  Each (m, bank) partial gets
                    # its own scratch accum col; the host combine sums
                    # them (the regular g0 cols stay unwritten for m<8).
                    for c in range(BANKS):
                        for q in range(2):
                            pt = pspool.tile([P, BANKS, 512], F32, tag="ps")
                            for i in range(4):
                                m = 4 * q + i
                                for kp in range(KP):
                                    nc.tensor.matmul(
                                        pt[:, i, 0:BANK],
                                        ht[:, kp, :, m * P:(m + 1) * P],
                                        wv[:, kp, :,
                                           c * BANK:(c + 1) * BANK],
                                        start=(kp == 0), stop=(kp == KP - 1),
                                        perf_mode=mybir.MatmulPerfMode
                                        .DoubleRow)
                            bbq = bb[:, c * BANK:(c + 1) * BANK] \
                                .rearrange("p (o c) -> p o c", o=1) \
                                .broadcast_to([P, 4, BANK])
                            jt = jpool.tile([P, BANKS, BANK], BF16, tag="jt")
                            nc.vector.tensor_tensor(
                                jt[:], pt[:, 0:BANKS, 0:BANK], bbq,
                                op=Alu.add)
                            et = epool.tile([P, BANKS, BANK], BF16, tag="et")
                            for i in range(4):
                                m = 4 * q + i
                                col = SC0 + m * 4 + c
                                nc.scalar.activation(
                                    et[:, i:i + 1, :], jt[:, i:i + 1, :],
                                    Act.Exp,
                                    accum_out=s_cols[:, col:col + 1])
                    continue
                bbv = bb[:, g * GV:(g + 1) * GV].rearrange(
                    "p (k c) -> p k c", c=BANK)
                for m in range(MBB):
                    last = b == NB - 1
                    if last and g == 1 and m >= MBB - 2:
                        # last two m-blocks: two 2-bank psum tiles each, so
                        # a pair's drain overlaps the next pair's matmuls
                        # (psum WAR tracking is tile-granular, so separate
                        # tiles are needed to overlap drain with fill) and
                        # the closing drain chain is half-size.  Second
                        # pairs accumulate into scratch cols SCL/SCL+1.
                        col0 = (b * MBB + m) * NG + g
                        scl = SCL + (MBB - 1 - m)
                        for hf in range(2):
                            pt = pspool.tile([P, BANKS, 512], F32, tag="ps")
                            for kp in range(KP):
                                for bk in range(2):
                                    nc.tensor.matmul(
                                        pt[:, bk, 0:BANK],
                                        ht[:, kp, :, m * P:(m + 1) * P],
                                        wv[:, kp, :,
                                           GV + (2 * hf + bk) * BANK:
                                           GV + (2 * hf + bk + 1) * BANK],
                                        start=(kp == 0), stop=(kp == KP - 1),
                                        perf_mode=mybir.MatmulPerfMode
                                        .DoubleRow)
                            bb2 = bb[:, GV + 2 * hf * BANK:
                                     GV + (2 * hf + 2) * BANK] \
                                .rearrange("p (k c) -> p k c", c=BANK)
                            jt = jpool.tile([P, 2, BANK], BF16, tag="jt2")
                            nc.vector.tensor_tensor(
                                jt[:], pt[:, 0:2, 0:BANK], bb2, op=Alu.add)
                            et = epool.tile([P, 2, BANK], BF16, tag="et2")
                            col = col0 if hf == 0 else scl
                            nc.scalar.activation(
                                et[:], jt[:], Act.Exp,
                                accum_out=s_cols[:, col:col + 1])
                        continue
                    pt = pspool.tile([P, BANKS, 512], F32, tag="ps")
                    for kp in range(KP):
                        lhsT = ht[:, kp, :, m * P:(m + 1) * P]
                        for bk in range(BANKS):
                            nc.tensor.matmul(
                                pt[:, bk, 0:BANK], lhsT,
                                wv[:, kp, :,
                                   g * GV + bk * BANK:g * GV + (bk + 1) * BANK],
                                start=(kp == 0), stop=(kp == KP - 1),
                                perf_mode=mybir.MatmulPerfMode.DoubleRow)
                    jt = jpool.tile([P, BANKS, BANK], BF16, tag="jt")
                    nc.vector.tensor_tensor(
                        jt[:], pt[:, 0:BANKS, 0:BANK], bbv, op=Alu.add)
                    et = epool.tile([P, BANKS, BANK], BF16, tag="et")
                    col = (b * MBB + m) * NG + g
                    nc.scalar.activation(
                        et[:], jt[:], Act.Exp,
                        accum_out=s_cols[:, col:col + 1])
                    # row-dot chunks in the drain slack.  Normally: muls of
                    # block b in b's g1 slots, reduces in b+1's g0 slots.
                    # The last block pulls both into its own slots so the
                    # tail has no row-dot work left.
                    last = b == NB - 1
                    if g == 0 and m < 4 and dj_prev is not None:
                        c = slice(m * DC, (m + 1) * DC)
                        nc.vector.tensor_reduce(
                            tpart[:, (b - 1) * 4 + m:(b - 1) * 4 + m + 1],
                            dj_prev[:, c], axis=mybir.AxisListType.X,
                            op=Alu.add)
                    if (g == 0 and 4 <= m if last else g == 1 and m < 4):
                        mm = m - 4 if last else m
                        c = slice(mm * DC, (mm + 1) * DC)
                        nc.vector.tensor_mul(dj[:, c], hg[:, c], wgt[:, c])
                    if last and g == 1 and m < 4:
                        c = slice(m * DC, (m + 1) * DC)
                        nc.vector.tensor_reduce(
                            tpart[:, b * 4 + m:b * 4 + m + 1],
                            dj[:, c], axis=mybir.AxisListType.X, op=Alu.add)
            dj_prev = dj
            ht_cur = ht_next

        # ship raw accumulator columns; host does the final folds
        nc.sync.dma_start(t_out[:], tpart[:])
        nc.sync.dma_start(s_out[:], s_cols[:])

    nc.compile()
    return nc


_NC_CACHE = {}


def _get_program():
    if "v2" not in _NC_CACHE:
        _NC_CACHE["v2"] = _build()
    return _NC_CACHE["v2"]


def kernel(hidden_states, head_weight, head_bias, loss_weight, labels,
           chunk_size=None, **_unused):
    hidden = np.asarray(hidden_states, dtype=np.float32)
    W = np.asarray(head_weight, dtype=np.float32)
    bias = np.asarray(head_bias, dtype=np.float32)
    lw = np.asarray(loss_weight, dtype=np.float32)
    labels = np.asarray(labels).astype(np.int64)

    assert hidden.shape == (N_TOK, D) and W.shape == (V, D)

    nc = _get_program()
    ht = np.ascontiguousarray(hidden.T)            # [D, N]
    Wt = np.ascontiguousarray(W.T)                 # [D, V]
    Wg = W[labels]                                 # gathered rows [N, D]
    in_maps = []
    for c in range(N_CORES):
        vsl = slice(c * VC, (c + 1) * VC)
        tsl = slice(c * TC, (c + 1) * TC)
        in_maps.append(dict(
            h=ht,
            W=np.ascontiguousarray(Wt[:, vsl]),
            bias=np.ascontiguousarray(bias[vsl]),
            hn=np.ascontiguousarray(hidden[tsl]),
            wg=np.ascontiguousarray(Wg[tsl])))
    res = run_bass_kernel_spmd(nc, in_maps, list(range(N_CORES)))

    # unshard + host-side scalar combine (the "all_reduce" of the hint):
    # fold the raw accumulator columns into per-token exp-sums, add the
    # vocab shards, then the weighted-mean reduction over tokens.
    sc = np.zeros((P, NSC), dtype=np.float64)
    for r in res.results:
        sc += r["s_out"].astype(np.float64)
    s = sc[:, 0:MG * NG].reshape(P, MG, NG).sum(-1)   # [P, MG]
    # block-0 g0 came as per-(m, bank) quarters (g0 slots unwritten there)
    s[:, 0:MBB] = sc[:, 1:2 * MBB:2] \
        + sc[:, SC0:SC0 + 4 * MBB].reshape(P, MBB, 4).sum(-1)
    s[:, MG - 1] += sc[:, SCL]                        # split-tile 2nd pairs
    s[:, MG - 2] += sc[:, SCL + 1]
    s = s.T.reshape(-1)                               # token-ordered [N]
    tgt = np.concatenate(
        [r["t_out"].astype(np.float64).reshape(P, MR, 4).sum(-1)
         .T.reshape(-1) for r in res.results])
    tgt = tgt + bias[labels].astype(np.float64)
    lse = np.log(s)
    nll = lse - tgt
    w64 = lw.astype(np.float64)
    loss = (w64 * nll).sum() / max(w64.sum(), 1.0)
    return np.float32(loss)


# revision 59
# speedup vs baseline: 1.0209x; 1.0004x over previous
"""Fused linear + cross-entropy loss (BaseChunkLoss) on 8 trn2 NeuronCores.

Strategy (vocab/tensor parallel, per the sharding hint's second option):
  - head_weight is sharded over vocab: each core owns a 4000-column slice
    of W (and bias) and computes, for ALL 8192 tokens, the partial
    sum-of-exp over its vocab slice.  The cross-device "logsumexp" is the
    host-side combine: s[tok] = sum_c s_c[tok], lse = log(s).
  - Per-core HBM traffic is ~3x lower than token-sharding (hidden 67MB +
    W slice 33MB vs. full W 262MB), which moves the kernel from DMA-bound
    to PE-bound at the fp8 DoubleRow matmul floor (~426us/core).
  - f32 -> fp8 conversion of both hidden and W happens inside the casting
    software-DGE DMA (gpsimd queue), so no staging buffers or on-chip
    conversion passes are needed.  W is cast unscaled; e4m3 subnormal
    rounding gives per-element absolute error comparable to the pre-scaled
    variant (measured end-to-end loss rel-err ~3e-5).
  - The target logit is computed exactly in f32 as a row-dot of the
    host-gathered rows W[labels] with hidden (token-sharded: each core
    does its own 1024 tokens), as mul+reduce chunks slotted into the
    DVE drain slack.  Host adds bias[labels] during the combine.

Device layout: tokens on PSUM partitions, vocab on the free dim.
  lhsT = hidden^T fp8 tile [128 d-pairs x 2 x 128 tok]  (stationary)
  rhs  = W^T      fp8 tile [128 d-pairs x 2 x 500 vocab] (moving)
  psum [128 tok x 4 banks x 500 vocab] f32, accumulated over D=2048 in 8
  DoubleRow steps (K=256 per matmul).
Drain per psum tile: DVE writes junk_bf16 = psum + bias (this frees the
psum bank pair in ~2.5us, under the 3.3us PE fill time, so PE never
stalls), then ACT computes exp(junk) with a fused row-sum accumulator
into s_cols, off the critical path.

Host-side prep is layout-only (transpose/slice/gather); all FLOPs over
hidden/weights happen on device inside the measured kernel.  The final
scalar combine (log, weighted mean, 8-way add) is the wrapper's
all_reduce stand-in.
"""
import numpy as np
from contextlib import ExitStack

from concourse import bacc, mybir, tile
from concourse.bass_utils import run_bass_kernel_spmd

F32 = mybir.dt.float32
BF16 = mybir.dt.bfloat16
FP8 = mybir.dt.float8e4
Alu = mybir.AluOpType
Act = mybir.ActivationFunctionType

N_CORES = 8
N_TOK = 8192
D = 2048
V = 32000
P = 128

VC = V // N_CORES       # 4000 vocab columns per core
GV = 2000               # vocab columns per drain group
NG = VC // GV           # 2 groups
BANKS = 4               # psum banks per group
BANK = GV // BANKS      # 500 vocab columns per bank
KP = D // 256           # 8 DoubleRow contraction steps (K=256 each)
TB = 1024               # tokens per streamed hidden block
NB = N_TOK // TB        # 8 blocks
MBB = TB // P           # 8 psum m-blocks per hidden block
MG = N_TOK // P         # 64 global m-blocks (s output columns)
TC = N_TOK // N_CORES   # 1024 tokens per core for the exact row-dot
MR = TC // P            # 8 row-dot m-blocks

# startup gpsimd DMA order: ("w"|"h", (kp0, kp1, lo, hi)) | ("b", (lo, hi))
# | ("r", 0) for the block-0 row-dot inputs.  Fine-grained so PE's first
# 16-matmul sweep (512 tokens x 500 vocab x kp0-3) starts after ~4us of
# DMA; every chunk keeps the per-descriptor contiguous run >= 512B (below
# that DMA pays 2x latency).
STARTUP_ORDER = (
    ("w", (0, 4, 0, 512)), ("h", (0, 4, 0, 512)), ("b", (0, 500)),
    ("w", (4, 8, 0, 512)), ("h", (4, 8, 0, 512)),
    ("h", (0, 4, 512, TB)), ("h", (4, 8, 512, TB)),
    ("w", (0, 4, 512, 1024)), ("w", (4, 8, 512, 1024)), ("b", (500, 1000)),
    ("w", (0, 4, 1024, GV)), ("w", (4, 8, 1024, GV)), ("b", (1000, GV)),
    ("w", (0, 4, GV, VC)), ("w", (4, 8, GV, VC)), ("b", (GV, VC)),
    ("r", 0),
)

SC0 = MG * NG           # scratch accum cols: 32 for block-0 g0 quarters
SCL = SC0 + 32          # + 3 for the last m-block's banks 1-3
NSC = SCL + 3


def _build():
    nc = bacc.Bacc("TRN2", target_bir_lowering=False, debug=False)
    # h and W arrive pre-transposed from host: h [D, N], W [D, VC]
    h_d = nc.declare_dram_parameter("h", [D, N_TOK], F32, isOutput=False)
    W_d = nc.declare_dram_parameter("W", [D, VC], F32, isOutput=False)
    bias_d = nc.declare_dram_parameter("bias", [VC], F32, isOutput=False)
    hn_d = nc.declare_dram_parameter("hn", [TC, D], F32, isOutput=False)
    wg_d = nc.declare_dram_parameter("wg", [TC, D], F32, isOutput=False)
    # raw accumulator columns; the host combine does the final folds
    s_out = nc.declare_dram_parameter("s_out", [P, NSC], F32, isOutput=True)
    t_out = nc.declare_dram_parameter("t_out", [P, MR * 4], F32, isOutput=True)

    # kp-major 4D views: one DMA can cover several kp slices at once
    # (SWDGE desc-gen costs ~1us + 0.34ns/desc per DMA instruction, so
    # fewer/bigger DMAs keep the Pool desc-gen chain off the critical path)
    W_q = W_d[:].rearrange("(kp j ki) v -> ki kp j v", ki=P, j=2)  # [128,8,2,VC]
    h_q = h_d[:].rearrange("(kp j ki) t -> ki kp j t", ki=P, j=2)  # [128,8,2,N]

    with tile.TileContext(nc) as tc, ExitStack() as ctx:
        wpool = ctx.enter_context(tc.tile_pool(name="w", bufs=1))
        hpool = ctx.enter_context(tc.tile_pool(name="hT", bufs=2))
        bpool = ctx.enter_context(tc.tile_pool(name="bias", bufs=1))
        gpool = ctx.enter_context(tc.tile_pool(name="gath", bufs=2))
        dpool = ctx.enter_context(tc.tile_pool(name="dj", bufs=1))
        jpool = ctx.enter_context(tc.tile_pool(name="jt", bufs=3))
        epool = ctx.enter_context(tc.tile_pool(name="et", bufs=2))
        pspool = ctx.enter_context(tc.tile_pool(name="ps", bufs=2, space="PSUM"))
        acc = ctx.enter_context(tc.tile_pool(name="acc", bufs=1))

        # cols 0:128 = (m, g) accumulators; 128:160 = block-0 first-group
        # per-(m, bank) quarter partials (g0 slots m*2 stay unwritten for
        # m<8 -- host uses the quarters instead); 160 = last-tile second
        # half.  tpart holds the 4 row-dot chunk partials per m-block.
        s_cols = acc.tile([P, NSC], F32, tag="scols")
        tpart = acc.tile([P, MR * 4], F32, tag="tpart")
        DC = D // 4             # row-dot chunk width

        wv = wpool.tile([P, KP, 2, VC], FP8, tag="w")
        bb = bpool.tile([P, VC], F32, tag="bias")

        KC = 4                  # kp slices per DMA (1024 descriptors)

        def load_h(b):
            ht = hpool.tile([P, KP, 2, TB], FP8, tag="hT")
            for k0 in range(0, KP, KC):
                nc.gpsimd.dma_start(
                    ht[:, k0:k0 + KC, :, :],
                    h_q[:, k0:k0 + KC, :, b * TB:(b + 1) * TB])
            return ht

        # Issue order on the gpsimd queue == DMA order.  The first psum
        # tile needs W group 0 + the first 128 tokens of hidden + bias
        # group 0; everything else overlaps with compute.
        ht_cur = hpool.tile([P, KP, 2, TB], FP8, tag="hT")

        def _dma_w(k0, k1, lo, hi):
            nc.gpsimd.dma_start(
                wv[:, k0:k1, :, lo:hi], W_q[:, k0:k1, :, lo:hi])

        def _dma_h0(k0, k1, lo, hi):
            nc.gpsimd.dma_start(
                ht_cur[:, k0:k1, :, lo:hi], h_q[:, k0:k1, :, lo:hi])

        def _dma_bias(lo, hi):
            nc.gpsimd.dma_start(
                bb[:, lo:hi], bias_d[lo:hi].partition_broadcast(P))

        hg0 = wgt0 = None
        for step in STARTUP_ORDER:
            kind, a = step
            if kind == "w":
                _dma_w(*a)
            elif kind == "h":
                _dma_h0(*a)
            elif kind == "b":
                _dma_bias(*a)
            else:
                hg0 = gpool.tile([P, D], F32, tag="hg")
                nc.gpsimd.dma_start(hg0[:], hn_d[0:P, :])
                wgt0 = gpool.tile([P, D], F32, tag="wg")
                nc.gpsimd.dma_start(wgt0[:], wg_d[0:P, :])

        def half_tile_pass(ht, pt, voff, msubs, cols, fuse=False):
            """Two 2-bank sub-tiles (one per (m, vocab-half)) sharing one
            4-bank psum tile: matmul sweeps first, then the drains.  With
            fuse=True (both subs over the SAME vocab half for different
            m-blocks) the bias-add runs as one 4-bank DVE op against a
            stride-0-repeated bias view, keeping the DVE drain rate equal
            to the main loop's."""
            def _mm_sub(i, m, bk0):
                for kp in range(KP):
                    lhsT = ht[:, kp, :, m * P:(m + 1) * P]
                    for bk in range(2):
                        nc.tensor.matmul(
                            pt[:, 2 * i + bk, 0:BANK], lhsT,
                            wv[:, kp, :,
                               voff + (bk0 + bk) * BANK:
                               voff + (bk0 + bk + 1) * BANK],
                            start=(kp == 0), stop=(kp == KP - 1),
                            perf_mode=mybir.MatmulPerfMode.DoubleRow)

            def _drain_sub(i, cols):
                bk0 = msubs[i][1]
                bb2 = bb[:, voff + bk0 * BANK:voff + (bk0 + 2) * BANK]
                jt = jpool.tile([P, 2, BANK], BF16, tag="jt2")
                nc.vector.tensor_tensor(
                    jt[:], pt[:, 2 * i:2 * i + 2, 0:BANK],
                    bb2.rearrange("p (k c) -> p k c", c=BANK), op=Alu.add)
                et = epool.tile([P, 2, BANK], BF16, tag="et2")
                nc.scalar.activation(
                    et[:], jt[:], Act.Exp,
                    accum_out=s_cols[:, cols[i]:cols[i] + 1])

            for i, (m, bk0) in enumerate(msubs):
                _mm_sub(i, m, bk0)
            if not fuse:
                _drain_sub(0, cols)
                _drain_sub(1, cols)
                return
            if fuse:
                bk0 = msubs[0][1]
                bbv4 = bb[:, voff + bk0 * BANK:voff + (bk0 + 2) * BANK] \
                    .rearrange("p (o k c) -> p o k c", o=1, c=BANK) \
                    .broadcast_to([P, 2, 2, BANK])
                jt = jpool.tile([P, BANKS, BANK], BF16, tag="jt")
                nc.vector.tensor_tensor(
                    jt[:].rearrange("p (a k) c -> p a k c", a=2),
                    pt[:, 0:BANKS, 0:BANK].rearrange(
                        "p (a k) c -> p a k c", a=2),
                    bbv4, op=Alu.add)
                et = epool.tile([P, BANKS, BANK], BF16, tag="et")
                for i in range(2):
                    nc.scalar.activation(
                        et[:, 2 * i:2 * i + 2, :], jt[:, 2 * i:2 * i + 2, :],
                        Act.Exp, accum_out=s_cols[:, cols[i]:cols[i] + 1])

        # Exact-f32 target-logit row-dot, chopped into D/4-wide chunks that
        # slot into the per-drain DVE slack (PE fills a psum tile in 3.33us,
        # the drain takes 2.2us; each 0.7us chunk fits the gap).  The
        # multiply chunks for block b run during b's g1 drains; the reduce
        # chunks run during block b+1's g0 drains (half-block lag so the
        # hg/wg DMAs always arrive in time).
        dj_prev = None
        for b in range(NB):
            ht = ht_cur
            if b == 0:
                hg, wgt = hg0, wgt0
            else:
                hg = gpool.tile([P, D], F32, tag="hg")
                nc.gpsimd.dma_start(hg[:], hn_d[b * P:(b + 1) * P, :])
                wgt = gpool.tile([P, D], F32, tag="wg")
                nc.gpsimd.dma_start(wgt[:], wg_d[b * P:(b + 1) * P, :])
            ht_next = load_h(b + 1) if b + 1 < NB else None
            dj = dpool.tile([P, D], F32, tag="dj")

            for g in range(NG):
                if b == 0 and g == 0:
                    # Block 0's first group runs as quarter tiles: one psum
                    # tile hosts 4 m-blocks x 1 bank (500 vocab cols), so
                    # the first matmul sweep needs only the first 512 W
                    # columns + 512 tokens.  Each (m, bank) partial gets
                    # its own scratch accum col; the host combine sums
                    # them (the regular g0 cols stay unwritten for m<8).
                    for c in range(BANKS):
                        for q in range(2):
                            pt = pspool.tile([P, BANKS, 512], F32, tag="ps")
                            for i in range(4):
                                m = 4 * q + i
                                for kp in range(KP):
                                    nc.tensor.matmul(
                                        pt[:, i, 0:BANK],
                                        ht[:, kp, :, m * P:(m + 1) * P],
                                        wv[:, kp, :,
                                           c * BANK:(c + 1) * BANK],
                                        start=(kp == 0), stop=(kp == KP - 1),
                                        perf_mode=mybir.MatmulPerfMode
                                        .DoubleRow)
                            bbq = bb[:, c * BANK:(c + 1) * BANK] \
                                .rearrange("p (o c) -> p o c", o=1) \
                                .broadcast_to([P, 4, BANK])
                            jt = jpool.tile([P, BANKS, BANK], BF16, tag="jt")
                            nc.vector.tensor_tensor(
                                jt[:], pt[:, 0:BANKS, 0:BANK], bbq,
                                op=Alu.add)
                            et = epool.tile([P, BANKS, BANK], BF16, tag="et")
                            for i in range(4):
                                m = 4 * q + i
                                col = SC0 + m * 4 + c
                                nc.scalar.activation(
                                    et[:, i:i + 1, :], jt[:, i:i + 1, :],
                                    Act.Exp,
                                    accum_out=s_cols[:, col:col + 1])
                    continue
                bbv = bb[:, g * GV:(g + 1) * GV].rearrange(
                    "p (k c) -> p k c", c=BANK)
                for m in range(MBB):
                    last = b == NB - 1
                    if last and g == 1 and m >= MBB - 2:
                        # last two m-blocks in sub-tiles, so a piece's
                        # drain overlaps the next piece's matmuls (psum WAR
                        # tracking is tile-granular, so separate tiles are
                        # needed to overlap drain with fill) and the
                        # closing drain chain shrinks.  m6 = two 2-bank
                        # pairs; m7 = pair + 1-bank + 1-bank.  Non-first
                        # pieces accumulate into scratch cols.
                        col0 = (b * MBB + m) * NG + g
                        if m == MBB - 2:
                            pieces = [(0, 2, col0), (2, 2, SCL)]
                        else:
                            pieces = [(0, 2, col0), (2, 1, SCL + 1),
                                      (3, 1, SCL + 2)]
                        for bk0, nb, col in pieces:
                            pt = pspool.tile([P, BANKS, 512], F32, tag="ps")
                            for kp in range(KP):
                                for bk in range(nb):
                                    nc.tensor.matmul(
                                        pt[:, bk, 0:BANK],
                                        ht[:, kp, :, m * P:(m + 1) * P],
                                        wv[:, kp, :,
                                           GV + (bk0 + bk) * BANK:
                                           GV + (bk0 + bk + 1) * BANK],
                                        start=(kp == 0), stop=(kp == KP - 1),
                                        perf_mode=mybir.MatmulPerfMode
                                        .DoubleRow)
                            bb2 = bb[:, GV + bk0 * BANK:
                                     GV + (bk0 + nb) * BANK] \
                                .rearrange("p (k c) -> p k c", c=BANK)
                            jt = jpool.tile([P, 2, BANK], BF16, tag="jt2")
                            nc.vector.tensor_tensor(
                                jt[:, 0:nb, :], pt[:, 0:nb, 0:BANK], bb2,
                                op=Alu.add)
                            et = epool.tile([P, 2, BANK], BF16, tag="et2")
                            nc.scalar.activation(
                                et[:, 0:nb, :], jt[:, 0:nb, :], Act.Exp,
                                accum_out=s_cols[:, col:col + 1])
                        continue
                    pt = pspool.tile([P, BANKS, 512], F32, tag="ps")
                    for kp in range(KP):
                        lhsT = ht[:, kp, :, m * P:(m + 1) * P]
                        for bk in range(BANKS):
                            nc.tensor.matmul(
                                pt[:, bk, 0:BANK], lhsT,
                                wv[:, kp, :,
                                   g * GV + bk * BANK:g * GV + (bk + 1) * BANK],
                                start=(kp == 0), stop=(kp == KP - 1),
                                perf_mode=mybir.MatmulPerfMode.DoubleRow)
                    jt = jpool.tile([P, BANKS, BANK], BF16, tag="jt")
                    nc.vector.tensor_tensor(
                        jt[:], pt[:, 0:BANKS, 0:BANK], bbv, op=Alu.add)
                    et = epool.tile([P, BANKS, BANK], BF16, tag="et")
                    col = (b * MBB + m) * NG + g
                    nc.scalar.activation(
                        et[:], jt[:], Act.Exp,
                        accum_out=s_cols[:, col:col + 1])
                    # row-dot chunks in the drain slack.  Normally: muls of
                    # block b in b's g1 slots, reduces in b+1's g0 slots.
                    # The last block pulls both into its own slots so the
                    # tail has no row-dot work left.
                    last = b == NB - 1
                    if g == 0 and m < 4 and dj_prev is not None:
                        c = slice(m * DC, (m + 1) * DC)
                        nc.vector.tensor_reduce(
                            tpart[:, (b - 1) * 4 + m:(b - 1) * 4 + m + 1],
                            dj_prev[:, c], axis=mybir.AxisListType.X,
                            op=Alu.add)
                    if (g == 0 and 4 <= m if last else g == 1 and m < 4):
                        mm = m - 4 if last else m
                        c = slice(mm * DC, (mm + 1) * DC)
                        nc.vector.tensor_mul(dj[:, c], hg[:, c], wgt[:, c])
                    if last and g == 1 and m < 4:
                        c = slice(m * DC, (m + 1) * DC)
                        nc.vector.tensor_reduce(
                            tpart[:, b * 4 + m:b * 4 + m + 1],
                            dj[:, c], axis=mybir.AxisListType.X, op=Alu.add)
            dj_prev = dj
            ht_cur = ht_next

        # ship raw accumulator columns; host does the final folds
        nc.sync.dma_start(t_out[:], tpart[:])
        nc.sync.dma_start(s_out[:], s_cols[:])

    nc.compile()
    return nc


_NC_CACHE = {}


def _get_program():
    if "v2" not in _NC_CACHE:
        _NC_CACHE["v2"] = _build()
    return _NC_CACHE["v2"]


def kernel(hidden_states, head_weight, head_bias, loss_weight, labels,
           chunk_size=None, **_unused):
    hidden = np.asarray(hidden_states, dtype=np.float32)
    W = np.asarray(head_weight, dtype=np.float32)
    bias = np.asarray(head_bias, dtype=np.float32)
    lw = np.asarray(loss_weight, dtype=np.float32)
    labels = np.asarray(labels).astype(np.int64)

    assert hidden.shape == (N_TOK, D) and W.shape == (V, D)

    nc = _get_program()
    ht = np.ascontiguousarray(hidden.T)            # [D, N]
    Wt = np.ascontiguousarray(W.T)                 # [D, V]
    Wg = W[labels]                                 # gathered rows [N, D]
    in_maps = []
    for c in range(N_CORES):
        vsl = slice(c * VC, (c + 1) * VC)
        tsl = slice(c * TC, (c + 1) * TC)
        in_maps.append(dict(
            h=ht,
            W=np.ascontiguousarray(Wt[:, vsl]),
            bias=np.ascontiguousarray(bias[vsl]),
            hn=np.ascontiguousarray(hidden[tsl]),
            wg=np.ascontiguousarray(Wg[tsl])))
    res = run_bass_kernel_spmd(nc, in_maps, list(range(N_CORES)))

    # unshard + host-side scalar combine (the "all_reduce" of the hint):
    # fold the raw accumulator columns into per-token exp-sums, add the
    # vocab shards, then the weighted-mean reduction over tokens.
    sc = np.zeros((P, NSC), dtype=np.float64)
    for r in res.results:
        sc += r["s_out"].astype(np.float64)
    s = sc[:, 0:MG * NG].reshape(P, MG, NG).sum(-1)   # [P, MG]
    # block-0 g0 came as per-(m, bank) quarters (g0 slots unwritten there)
    s[:, 0:MBB] = sc[:, 1:2 * MBB:2] \
        + sc[:, SC0:SC0 + 4 * MBB].reshape(P, MBB, 4).sum(-1)
    s[:, MG - 1] += sc[:, SCL + 1] + sc[:, SCL + 2]   # split-tile pieces
    s[:, MG - 2] += sc[:, SCL]
    s = s.T.reshape(-1)                               # token-ordered [N]
    tgt = np.concatenate(
        [r["t_out"].astype(np.float64).reshape(P, MR, 4).sum(-1)
         .T.reshape(-1) for r in res.results])
    tgt = tgt + bias[labels].astype(np.float64)
    lse = np.log(s)
    nll = lse - tgt
    w64 = lw.astype(np.float64)
    loss = (w64 * nll).sum() / max(w64.sum(), 1.0)
    return np.float32(loss)
